# revision 43
# baseline (speedup 1.0000x reference)
"""Dense transformer block on 8 Trainium2 NeuronCores.

Sharding: each core owns half a batch element (512 rows out of [4, 1024, C]).
Cores redundantly compute LN1 + K/V projections for the full batch element
(so attention needs no cross-core communication); Q / attention / proj / MLP
run only on the core's own 512 rows.  No collectives.

Host-side prep:
  - all weights pre-transposed to [in_feat, out_feat], cast to bf16
  - LN gammas folded into the following matmul weights, betas into biases
  - q scale (1/sqrt(hd)) folded into W_q / b_q
  - k bias dropped (softmax shift-invariant), v bias folded into proj bias
    (softmax rows sum to 1)
  - per-core x rows permuted to [own 512 | other 512]; softmax is invariant
    to key/value ordering so attention over permuted K/V is exact.

On-chip layout notes:
  - projection/MLP matmuls in bf16 (same PE rate as fp32r at free>=256 but
    half the DMA/SBUF and 1 cyc/row transposes); scores in fp32r; PSUM fp32
  - LN stats on DVE, the elementwise apply on the scalar engine as
    Identity(x*rstd + (-mu*rstd)) -- DVE writes of bf16 take a slow path
  - per x-tile the 8 PE transposes land in one PSUM tile and move to SBUF
    with a single strided copy (alternating scalar/vector)
  - softmax denominators gathered into per-batch tiles via tiny SBUF->SBUF
    DMAs, one batched DVE reciprocal per batch (partition-shifted compute
    ops must be 32-aligned; DMA is not), broadcast back over a DRAM
    round-trip, and multiplied into oT on gpsimd (SBUF-only engine)
  - proj/fc2 biases accumulated into PSUM via rank-1 ones x bias matmuls
  - weight streams split across DMA queues (sync: wv/wqk/wp/fc2, gpsimd:
    fc1) so no stream queues behind another; fc2 weights fully prefetched
    and fc2 iterated row-tile-outermost so the epilogue overlaps compute
"""

import sys

if "/opt/trn_rl_repo" not in sys.path:
    sys.path.insert(0, "/opt/trn_rl_repo")

import ml_dtypes
import numpy as np

import concourse.bacc as bacc
import concourse.bass as bass
import concourse.mybir as mybir
import concourse.tile as tile
from concourse.masks import make_identity
from concourse.tile_rust import add_dep_helper
FP = mybir.dt.float32
FPR = mybir.dt.float32r  # bit-identical to fp32; copies with partition shift
BF = mybir.dt.bfloat16
AF = mybir.ActivationFunctionType
BF_NP = ml_dtypes.bfloat16

N_CORES = 8
P = 128
C = 1024            # model dim
H = 16              # heads
HD = 64             # head dim
HID = 4096          # mlp hidden
N_ALL = 1024        # rows per batch element
N_OWN = 512         # rows owned per core
EPS = 1e-5

CT = C // P         # 8 feature chunks of 128
NT_ALL = N_ALL // P # 8 row tiles
NT_OWN = N_OWN // P # 4 row tiles


def _mm(nc, out, lhsT, rhs, start, stop):
    nc.tensor.matmul(out, lhsT, rhs, start=start, stop=stop)


def _bcast_row(ap, parts):
    """Partition-broadcast a [free] DRAM AP to [parts, free] (step-0 dim)."""
    return bass.AP(tensor=ap.tensor, offset=ap.offset,
                   ap=[[0, parts], *list(ap.ap)])


def _layernorm_tile(nc, pool, out, xt, eps_sb, neg1):
    """out(bf16) = (xt - mean) * rsqrt(var + eps), rows of width C.

    Stats on vector; the elementwise apply runs on the scalar engine as
    out = Copy(x * rstd + (-mean * rstd)) with per-partition scale/bias
    (DVE writes of bf16 take a slow path; scalar ACT does the cast free).
    """
    st = pool.tile([P, 2, 6], FP, tag="ln_st", name="ln_st")
    for g in range(2):
        nc.vector.bn_stats(out=st[:, g, :], in_=xt[:, 512 * g:512 * (g + 1)])
    mv = pool.tile([P, 2], FP, tag="ln_mv", name="ln_mv")
    nc.vector.bn_aggr(out=mv, in_=st)
    rstd = pool.tile([P, 1], FP, tag="ln_rstd", name="ln_rstd")
    nc.scalar.activation(out=rstd, in_=mv[:, 1:2], func=AF.Sqrt, bias=eps_sb, scale=1.0)
    nc.vector.reciprocal(out=rstd, in_=rstd)
    nmr = pool.tile([P, 1], FP, tag="ln_nmr", name="ln_nmr")
    nc.vector.tensor_scalar(
        out=nmr, in0=mv[:, 0:1], scalar1=rstd, scalar2=neg1,
        op0=mybir.AluOpType.mult, op1=mybir.AluOpType.mult,
    )
    nc.scalar.activation(out=out, in_=xt, func=AF.Identity,
                         bias=nmr, scale=rstd)


def build():
    nc = bacc.Bacc("TRN2", target_bir_lowering=False, debug=False,
                   num_devices=N_CORES)

    x_d = nc.dram_tensor("x", [N_ALL, C], BF, kind="ExternalInput")
    wqk_d = nc.dram_tensor("wqk", [CT * C, 256], BF, kind="ExternalInput")
    wv_d = nc.dram_tensor("wv", [C, C], BF, kind="ExternalInput")
    bq_d = nc.dram_tensor("bq", [C], FP, kind="ExternalInput")
    wp_d = nc.dram_tensor("wp", [C, C], BF, kind="ExternalInput")
    bp_d = nc.dram_tensor("bp", [1, C], BF, kind="ExternalInput")
    w2_d = nc.dram_tensor("w2", [8 * C, 512], BF, kind="ExternalInput")
    b2_d = nc.dram_tensor("b2", [HID], FP, kind="ExternalInput")
    wf2_d = nc.dram_tensor("wf2", [HID, C], BF, kind="ExternalInput")
    bf2_d = nc.dram_tensor("bf2", [1, C], BF, kind="ExternalInput")
    out_d = nc.dram_tensor("out", [N_OWN, C], FP, kind="ExternalOutput")

    with tile.TileContext(nc, pool_alloc_mode="queue") as tc:
        consts = tc.alloc_tile_pool(name="consts", bufs=1)
        ident = consts.tile([P, P], BF)
        make_identity(nc, ident)
        bq_sb = consts.tile([P, CT], FP)
        nc.sync.dma_start(out=bq_sb, in_=bq_d[:].rearrange("(j p) -> p j", p=P))
        b2_sb = consts.tile([P, HID // P], FP)
        nc.sync.dma_start(out=b2_sb, in_=b2_d[:].rearrange("(j p) -> p j", p=P))
        bp_row = consts.tile([1, C], BF)
        nc.sync.dma_start(out=bp_row, in_=bp_d[:, :])
        bf2_row = consts.tile([1, C], BF)
        nc.sync.dma_start(out=bf2_row, in_=bf2_d[:, :])
        eps_sb = consts.tile([P, 1], FP)
        nc.vector.memset(eps_sb, EPS)
        ones_row = consts.tile([1, P], BF)
        nc.vector.memset(ones_row, 1.0)
        neg1 = consts.tile([P, 1], FP)
        nc.vector.memset(neg1, -1.0)

        # ---- Phase A: load x, LN1, transpose h -> hT [C, N_ALL] (bf16) ----
        hT_pool = tc.alloc_tile_pool(name="hT", bufs=1)
        wV = tc.alloc_tile_pool(name="wV", bufs=16)
        pa = tc.alloc_tile_pool(name="pa", bufs=4)
        ha = tc.alloc_tile_pool(name="ha", bufs=2)
        psA = tc.alloc_tile_pool(name="psA", bufs=3, space="PSUM")
        hT_all = hT_pool.tile([P, CT, N_ALL], BF, tag="hT", name="hT")
        hT = [hT_all[:, c, :] for c in range(CT)]
        x_tiles = []
        x_dmas = []
        for i in range(NT_ALL):
            xt = pa.tile([P, C], BF, tag="x_in", name="x_in")
            for hh in range(2):
                dma = nc.gpsimd.dma_start(
                    out=xt[64 * hh:64 * (hh + 1), :],
                    in_=x_d[P * i + 64 * hh:P * i + 64 * (hh + 1), :])
                x_dmas.append(dma)
            x_tiles.append(xt)

        # V weights: first on the sync queue (deferred until x is mostly in,
        # so the x load gets the full HBM read bandwidth)
        wv_t = {}
        for grp in range(2):
            for c in range(CT):
                w = wV.tile([P, 512], BF, tag="wV", name="wV")
                dma = nc.sync.dma_start(
                    out=w, in_=wv_d[P * c:P * (c + 1), 512 * grp:512 * (grp + 1)])
                if grp == 0 and c == 0:
                    add_dep_helper(dma.ins, x_dmas[3].ins, sync=True,
                                   reason="defer weight stream behind x load")
                wv_t[(grp, c)] = w

        for i in range(NT_ALL):
            xt = x_tiles[i]
            ht = ha.tile([P, C], BF, tag="h", name="h")
            _layernorm_tile(nc, pa, ht, xt, eps_sb, neg1)
            ps = psA.tile([P, C], BF, tag="psA", name="psA")
            for j in range(CT):
                nc.tensor.transpose(ps[:, P * j:P * (j + 1)],
                                    ht[:, P * j:P * (j + 1)], ident)
            if i % 2 == 0:
                nc.vector.tensor_copy(
                    out=hT_all[:, :, P * i:P * (i + 1)],
                    in_=ps.rearrange("p (j c) -> p j c", j=CT))
            else:
                nc.scalar.activation(
                    out=hT_all[:, :, P * i:P * (i + 1)],
                    in_=ps.rearrange("p (j c) -> p j c", j=CT), func=AF.Copy)
        ha.release()
        pa.release()
        psA.release()

        # ---- Phase B: v projection -> v_aug [keys, H, HD+1] (bf16) ----
        v_pool = tc.alloc_tile_pool(name="v", bufs=NT_ALL, side="right")
        psV = tc.alloc_tile_pool(name="psV", bufs=4, space="PSUM")
        v_aug = [v_pool.tile([P, H, HD + 1], BF, tag="v", name="v") for _ in range(NT_ALL)]
        for m in range(NT_ALL):
            nc.vector.memset(v_aug[m][:, :, HD:HD + 1], 1.0)
        for grp in range(2):
            for m in range(NT_ALL):
                ps = psV.tile([P, 512], FP, tag="psV", name="psV")
                for c in range(CT):
                    _mm(nc, ps, hT[c][:, P * m:P * (m + 1)], wv_t[(grp, c)],
                        c == 0, c == CT - 1)
                h0 = grp * 8
                nc.scalar.activation(
                    out=v_aug[m][:, h0:h0 + 8, 0:HD],
                    in_=ps.rearrange("p (h d) -> p h d", h=8), func=AF.Copy)
        psV.release()
        wV.release()

        # right stack: attention outputs (live until proj/fc2)
        oT_pool = tc.alloc_tile_pool(name="oT", bufs=CT, side="right")
        den_pool = tc.alloc_tile_pool(name="den", bufs=1, side="right")
        oT = [oT_pool.tile([P, N_OWN], BF, tag="oT", name="oT") for _ in range(CT)]
        DEN_BATCHES = ((0, 8), (8, 6), (14, 2))   # (first head, count)
        den_all = [den_pool.tile([n_, N_OWN], FPR, tag=f"den_all{b}",
                                 name=f"den_all{b}")
                   for b, (_, n_) in enumerate(DEN_BATCHES)]

        def _den_batch_of(h):
            for b, (h0, n_) in enumerate(DEN_BATCHES):
                if h0 <= h < h0 + n_:
                    return b, h - h0
            raise AssertionError(h)

        wqk = tc.alloc_tile_pool(name="wqk", bufs=12)
        qT_pool = tc.alloc_tile_pool(name="qT", bufs=2)
        kT_pool = tc.alloc_tile_pool(name="kT", bufs=2)
        pt_pool = tc.alloc_tile_pool(name="pt", bufs=8)
        bc_pool = tc.alloc_tile_pool(name="bc", bufs=2)
        oTf_pool = tc.alloc_tile_pool(name="oTf", bufs=5)
        dr_pool = tc.alloc_tile_pool(name="dr", bufs=2)
        dd_pool = tc.alloc_tile_pool(name="dd", bufs=2, space="DRAM")
        psS = tc.alloc_tile_pool(name="psS", bufs=3, space="PSUM")
        psO = tc.alloc_tile_pool(name="psO", bufs=2, space="PSUM")

        oTf_t = {}

        def _norm_heads(b):
            """Batched reciprocal of a denominator batch; bcast + scale oT.

            The fp32 staging tile oTf is scaled by the bcast reciprocal and
            cast to bf16 in a single gpsimd tensor_mul (SBUF-only engine).
            """
            h0, n_ = DEN_BATCHES[b]
            nc.vector.reciprocal(out=den_all[b].bitcast(FP),
                                 in_=den_all[b].bitcast(FP))
            dd = dd_pool.tile([8, N_OWN], FP, tag="dd", name="dd")
            nc.gpsimd.dma_start(out=dd[0:n_, :], in_=den_all[b].bitcast(FP))
            for h in range(h0, h0 + n_):
                p_, odd = divmod(h, 2)
                half = slice(HD * odd, HD * (odd + 1))
                bc = bc_pool.tile([P, N_OWN], FP, tag="bc", name="bc")
                nc.gpsimd.dma_start(out=bc[half, :],
                                    in_=_bcast_row(dd[h - h0, :], HD))
                nc.gpsimd.tensor_mul(out=oT[p_][half, :], in0=oTf_t[p_][half, :],
                                     in1=bc[half, :])

        for p in range(CT):            # head pairs
            ws = [wqk.tile([P, 256], BF, tag="wqk", name="wqk") for _ in range(CT)]
            for c in range(CT):
                nc.sync.dma_start(
                    out=ws[c], in_=wqk_d[C * p + P * c:C * p + P * (c + 1), :])
            qTp = qT_pool.tile([P, N_OWN], FPR, tag="qT", name="qT")
            kTp = kT_pool.tile([P, N_ALL], FPR, tag="kT", name="kT")
            ps = psS.tile([P, 1024], FP, tag="psS", name="psS")
            for c in range(CT):
                _mm(nc, ps[:, 0:512], ws[c][:, 0:P], hT[c][:, 0:N_OWN],
                    c == 0, c == CT - 1)
            nc.vector.tensor_scalar_add(out=qTp, in0=ps[:, 0:512],
                                        scalar1=bq_sb[:, p:p + 1])
            for s in range(2):
                ps = psS.tile([P, 1024], FP, tag="psS", name="psS")
                for c in range(CT):
                    _mm(nc, ps[:, 512 * s:512 * (s + 1)], ws[c][:, P:256],
                        hT[c][:, 512 * s:512 * (s + 1)], c == 0, c == CT - 1)
                nc.vector.tensor_copy(out=kTp[:, 512 * s:512 * (s + 1)],
                                       in_=ps[:, 512 * s:512 * (s + 1)])

            for odd in range(2):
                h = 2 * p + odd
                kt = kTp[HD * odd:HD * (odd + 1), :]
                qt = qTp[HD * odd:HD * (odd + 1), :]
                pts = []
                for t in range(4):
                    ps = psS.tile([P, 1024], FP, tag="psS", name="psS")
                    _mm(nc, ps[:, 0:512], kt[:, P * 2 * t:P * (2 * t + 1)], qt,
                        True, True)
                    _mm(nc, ps[:, 512:1024], kt[:, P * (2 * t + 1):P * (2 * t + 2)],
                        qt, True, True)
                    pt = pt_pool.tile([P, 1024], BF, tag="pt", name="pt")
                    nc.scalar.activation(out=pt, in_=ps, func=AF.Exp)
                    pts.append(pt)
                po = psO.tile([HD + 1, N_OWN], FP, tag="psO", name="psO")
                for t in range(4):
                    _mm(nc, po, v_aug[2 * t][:, h, :], pts[t][:, 0:512],
                        t == 0, False)
                    _mm(nc, po, v_aug[2 * t + 1][:, h, :], pts[t][:, 512:1024],
                        False, t == 3)
                half = slice(HD * odd, HD * (odd + 1))
                if odd == 0:
                    oTf_t[p] = oTf_pool.tile([P, N_OWN], FPR, tag="oTf", name="oTf")
                nc.vector.tensor_copy(out=oTf_t[p][half, :], in_=po[0:HD, :])
                dr = dr_pool.tile([1, N_OWN], FPR, tag="dr", name="dr")
                nc.vector.tensor_copy(out=dr, in_=po[HD:HD + 1, :])
                db, di = _den_batch_of(h)
                nc.gpsimd.dma_start(out=den_all[db][di:di + 1, :], in_=dr)
            if p == 3:
                _norm_heads(0)
            if p == 6:
                _norm_heads(1)
            if p == 5:
                # prefetch proj weights + first fc1 groups during attn tail
                # (right stack, above den_pool; released after fc1/proj)
                wD = tc.alloc_tile_pool(name="wD", bufs=CT, side="right")
                wp_t = [wD.tile([P, C], BF, tag="wD", name="wD") for _ in range(CT)]
                for c in range(CT):
                    nc.sync.dma_start(out=wp_t[c], in_=wp_d[P * c:P * (c + 1), :])
                wF = tc.alloc_tile_pool(name="wF", bufs=14, side="right")
                wf_tiles = {}
                for g in range(2):
                    for c in range(CT):
                        w = wF.tile([P, 512], BF, tag="wF", name="wF")
                        nc.sync.dma_start(
                            out=w,
                            in_=w2_d[C * g + P * c:C * g + P * (c + 1), :])
                        wf_tiles[(0, g, c)] = w
        _norm_heads(2)

        # stack pops at attention end (reverse alloc order)
        dd_pool.release()
        dr_pool.release()
        oTf_pool.release()
        bc_pool.release()
        pt_pool.release()
        kT_pool.release()
        qT_pool.release()
        wqk.release()
        hT_pool.release()
        psO.release()
        psS.release()

        # stream all fc2 weights now; they are needed from fc1-end onward
        wGa = tc.alloc_tile_pool(name="wGa", bufs=15, side="right")
        wGb = tc.alloc_tile_pool(name="wGb", bufs=17, side="right")
        wg_t = []
        for hf in range(HID // P):
            pool_ = wGa if hf < 15 else wGb
            wt = pool_.tile([P, C], BF, tag="wG", name="wG")
            nc.sync.dma_start(out=wt, in_=wf2_d[P * hf:P * (hf + 1), :])
            wg_t.append(wt)

        # ---- Phase D+E: proj + residual -> x2; LN2 + transpose -> h2T ----
        x2_pool = tc.alloc_tile_pool(name="x2", bufs=NT_OWN)
        h2T_pool = tc.alloc_tile_pool(name="h2T", bufs=1)
        xr_pool = tc.alloc_tile_pool(name="xr", bufs=NT_OWN)
        pe = tc.alloc_tile_pool(name="pe", bufs=4)
        he = tc.alloc_tile_pool(name="he", bufs=NT_OWN)
        xr = [xr_pool.tile([P, C], BF, tag="xr", name="xr") for _ in range(NT_OWN)]
        for n in range(NT_OWN):
            nc.scalar.dma_start(out=xr[n], in_=x_d[P * n:P * (n + 1), :])
        psD = tc.alloc_tile_pool(name="psD", bufs=4, space="PSUM")
        psE = tc.alloc_tile_pool(name="psE", bufs=2, space="PSUM")
        h2T_all = h2T_pool.tile([P, CT, N_OWN], BF, tag="h2T", name="h2T")
        h2T = [h2T_all[:, c, :] for c in range(CT)]
        x2 = [x2_pool.tile([P, C], FP, tag="x2", name="x2") for _ in range(NT_OWN)]
        ht2 = []
        for n in range(NT_OWN):
            pss = [psD.tile([P, 512], FP, tag="psD", name="psD") for _ in range(2)]
            for cc in range(2):
                _mm(nc, pss[cc], ones_row, bp_row[:, 512 * cc:512 * (cc + 1)],
                    True, False)
            for of in range(CT):
                for cc in range(2):
                    _mm(nc, pss[cc], oT[of][:, P * n:P * (n + 1)],
                        wp_t[of][:, 512 * cc:512 * (cc + 1)],
                        False, of == CT - 1)
            for cc in range(2):
                sl = slice(512 * cc, 512 * (cc + 1))
                nc.vector.tensor_add(out=x2[n][:, sl], in0=pss[cc],
                                     in1=xr[n][:, sl])
            ht = he.tile([P, C], BF, tag="h2", name="h2")
            _layernorm_tile(nc, pe, ht, x2[n], eps_sb, neg1)
            ht2.append(ht)
        # transposes in a second pass: the in-order PE queue never waits on
        # the n-th LN2 chain (it completed while proj n+1/n+2 ran)
        for n in range(NT_OWN):
            ps = psE.tile([P, C], BF, tag="psE", name="psE")
            for j in range(CT):
                nc.tensor.transpose(ps[:, P * j:P * (j + 1)],
                                    ht2[n][:, P * j:P * (j + 1)], ident)
            if n % 2 == 0:
                nc.vector.tensor_copy(
                    out=h2T_all[:, :, P * n:P * (n + 1)],
                    in_=ps.rearrange("p (j c) -> p j c", j=CT))
            else:
                nc.scalar.activation(
                    out=h2T_all[:, :, P * n:P * (n + 1)],
                    in_=ps.rearrange("p (j c) -> p j c", j=CT), func=AF.Copy)
        he.release()
        pe.release()
        xr_pool.release()
        psE.release()
        psD.release()

        # ---- Phase F: fc1 + gelu -> h3T [HID, N_OWN] (bf16) ----
        # Column halves: rows 0:256 (x2 tiles n=0,1) computed for every group
        # first, so fc1 starts as soon as half of LN2/transpose is done.
        h3Ta = tc.alloc_tile_pool(name="h3Ta", bufs=16, side="right")
        h3Tb = tc.alloc_tile_pool(name="h3Tb", bufs=16, side="right")
        psF = tc.alloc_tile_pool(name="psF", bufs=4, space="PSUM")
        h3T = [(h3Ta if i < 16 else h3Tb).tile([P, N_OWN], BF, tag="h3T",
                                               name="h3T")
               for i in range(HID // P)]
        for g in range(8):             # groups of 4 hf-tiles
            if g >= 2:
                for c in range(CT):
                    w = wF.tile([P, 512], BF, tag="wF", name="wF")
                    nc.gpsimd.dma_start(
                        out=w, in_=w2_d[C * g + P * c:C * g + P * (c + 1), :])
                    wf_tiles[(0, g, c)] = w
            for f in range(4):
                hf = 4 * g + f
                ps = psF.tile([P, 512], FP, tag="psF", name="psF")
                for c in range(CT):
                    _mm(nc, ps, wf_tiles[(0, g, c)][:, P * f:P * (f + 1)],
                        h2T[c], c == 0, c == CT - 1)
                nc.scalar.activation(out=h3T[hf], in_=ps, func=AF.Gelu,
                                     bias=b2_sb[:, hf:hf + 1], scale=1.0)
        h2T_pool.release()
        psF.release()

        # ---- Phase G: fc2 + residual -> out ----
        # Weights were fully prefetched into wG during proj/fc1; iterate n
        # outermost so each row tile finishes (adds + store) while the PE
        # works on the next one.
        psG = tc.alloc_tile_pool(name="psG", bufs=2, space="PSUM")
        out_pool = tc.alloc_tile_pool(name="outp", bufs=2)
        NHF = HID // P
        for n in range(NT_OWN):
            pgn = [psG.tile([P, 512], FP, tag=f"psG{cc}", name=f"psG{cc}")
                   for cc in range(2)]
            for cc in range(2):
                _mm(nc, pgn[cc], ones_row, bf2_row[:, 512 * cc:512 * (cc + 1)],
                    True, False)
            for hf in range(NHF):
                for cc in range(2):
                    _mm(nc, pgn[cc], h3T[hf][:, P * n:P * (n + 1)],
                        wg_t[hf][:, 512 * cc:512 * (cc + 1)],
                        False, hf == NHF - 1)
            x3 = out_pool.tile([P, C], FP, tag="x3", name="x3")
            for cc2 in range(2):
                sl = slice(512 * cc2, 512 * (cc2 + 1))
                nc.vector.tensor_add(out=x3[:, sl], in0=pgn[cc2],
                                     in1=x2[n][:, sl])
            nc.scalar.dma_start(out=out_d[P * n:P * (n + 1), :], in_=x3)
        out_pool.release()
        psG.release()
        # right stack pops
        h3Tb.release()
        h3Ta.release()
        wGb.release()
        wGa.release()
        wF.release()
        wD.release()
        den_pool.release()
        oT_pool.release()
        v_pool.release()
        # left stack pops
        x2_pool.release()
        consts.release()

    nc.compile()
    return nc


_NC = None


def _get_nc():
    global _NC
    if _NC is None:
        _NC = build()
    return _NC


def _prep(inputs):
    f32 = lambda a: np.ascontiguousarray(np.asarray(a, dtype=np.float32))
    bf16 = lambda a: np.ascontiguousarray(np.asarray(a, dtype=np.float32).astype(BF_NP))
    x = f32(inputs["x"])
    qkv_w, qkv_b = f32(inputs["qkv_w"]), f32(inputs["qkv_b"])
    proj_w, proj_b = f32(inputs["proj_w"]), f32(inputs["proj_b"])
    fc1_w, fc1_b = f32(inputs["fc1_w"]), f32(inputs["fc1_b"])
    fc2_w, fc2_b = f32(inputs["fc2_w"]), f32(inputs["fc2_b"])
    ln1_g, ln1_b = f32(inputs["ln1_g"]), f32(inputs["ln1_b"])
    ln2_g, ln2_b = f32(inputs["ln2_g"]), f32(inputs["ln2_b"])

    scale = np.float32(HD ** -0.5)
    w1 = (qkv_w * ln1_g[None, :]).T                 # [C, 3C]
    b1 = qkv_b + qkv_w @ ln1_b                      # [3C]
    wq = w1[:, :C] * scale
    wk = w1[:, C:2 * C]
    wv = np.ascontiguousarray(w1[:, 2 * C:])
    # wqk layout [8 pairs][C rows][256]: per pair p the 128 q cols then 128 k
    wqk = np.empty((CT, C, 256), dtype=np.float32)
    for p_ in range(CT):
        wqk[p_, :, 0:P] = wq[:, P * p_:P * (p_ + 1)]
        wqk[p_, :, P:256] = wk[:, P * p_:P * (p_ + 1)]
    wqk = wqk.reshape(CT * C, 256)
    bq = (b1[:C] * scale).copy()
    bv = b1[2 * C:]
    wp = proj_w.T.copy()                            # [C, C]
    bp = proj_b + proj_w @ bv
    w2 = (fc1_w * ln2_g[None, :]).T.copy()          # [C, HID]
    b2 = fc1_b + fc1_w @ ln2_b
    # w2 layout [8 groups][C rows][512]
    w2g = np.ascontiguousarray(
        w2.reshape(C, 8, 512).transpose(1, 0, 2)).reshape(8 * C, 512)
    wf2 = fc2_w.T.copy()                            # [HID, C]
    bf2 = fc2_b

    shared = dict(wqk=bf16(wqk), wv=bf16(wv), bq=f32(bq), wp=bf16(wp),
                  bp=bf16(bp.reshape(1, C)), w2=bf16(w2g), b2=f32(b2),
                  wf2=bf16(wf2), bf2=bf16(bf2.reshape(1, C)))
    in_maps = []
    for c in range(N_CORES):
        b, half = divmod(c, 2)
        own = x[b, N_OWN * half:N_OWN * (half + 1), :]
        oth = x[b, N_OWN * (1 - half):N_OWN * (2 - half), :]
        xp = np.concatenate([own, oth], axis=0)
        in_maps.append({"x": bf16(xp), **shared})
    return in_maps


def run(inputs, trace=False, trace_kwargs=None):
    from concourse.bass_utils import run_bass_kernel_spmd
    nc = _get_nc()
    in_maps = _prep(inputs)
    res = run_bass_kernel_spmd(nc, in_maps, core_ids=list(range(N_CORES)),
                               trace=trace, **(trace_kwargs or {}))
    B = 4
    out = np.empty((B, N_ALL, C), dtype=np.float32)
    for c in range(N_CORES):
        b, half = divmod(c, 2)
        out[b, N_OWN * half:N_OWN * (half + 1), :] = res.results[c]["out"]
    return out, res


def kernel(**inputs):
    out, _ = run(inputs, trace=False)
    return out
